# revision 7
# baseline (speedup 1.0000x reference)
"""BlockReLU Trainium2 kernel (v5).

Full input: activation [32, 128, 112, 112] f32. Channel groups:
  [0,64): 1x1 blocks (plain ReLU), [64,96): 2x2 blocks, [96,120): 4x4 blocks,
  [120,128): identity passthrough.
A block's mask is 1 where the block's spatial sum >= 0, else 0; broadcast over
the block and multiplied into the input.

Data-parallel over batch N across 8 cores (4 images/core), H streamed in
chunks. Traffic: G1 loads fp16 (rounding never flips sign(x) so the 1x1 mask
is exact), G2/G3 loads fp32 (lossy inputs would flip near-zero block-sum
signs; sum tree matches the reference bit-for-bit), all stores fp16 via
engine write-port conversion, identity channels handled host-side.
Per-core 29.7 MB vs 51.4 baseline. Measured per-core DMA ceiling ~420 GB/s
(16 engines x 26 GB/s, cost linear in packet size).

Scheduling (from v4 trace analysis):
  - Chunk taper [8,20,20,16,16,16,8,4,4]: small first chunk starts compute
    early, big chunks 1-2 cut descriptor-push count while the pipe fills,
    4-row last chunks drain the compute+store tail fast.
  - Ramp: chunks 0-2 G1 loads ride the otherwise-idle SWDGE ring, emitted
    BEFORE any store instruction so they are never head-of-line blocked by
    a store's semaphore wait; HWDGE rings then carry only x2/x3 early.
  - Tail (last 3 chunks): ALL remaining loads are emitted before any tail
    store per engine program (v4 put stores ahead of later loads on the
    same ring, serializing the tail); y2/y3 stores flush on the HWDGE
    rings, y1 on SWDGE - 3-way parallel drain.
  - Mid-stream: loads on both HWDGE rings (x2 vs x3+x1 split alternates by
    parity for byte balance), stores on SWDGE only.
DVE: mask apply per row-parity plane - 3D-collapsible APs, contiguous fp16
writes, stride-0 broadcast of the mask over the w-block (STT is limited to
3D; 4D/5D forms that don't collapse are rejected by the compiler).
History: v1 148/135 us, v2 (fp16 stores) 120, v3 (bf16 G1, no identity) 103,
v4 (taper, fp16 G1, plane STT) 98.8, v5 predicted ~90.
"""
import sys

if "/opt/trn_rl_repo" not in sys.path:
    sys.path.insert(0, "/opt/trn_rl_repo")

import numpy as np
from contextlib import ExitStack

import concourse.tile as tile
from concourse import bacc, mybir
from concourse.bass_utils import run_bass_kernel_spmd

N_FULL, C, H, W = 32, 128, 112, 112
C_OUT = 120
N_CORES = 8
N_PER_CORE = N_FULL // N_CORES  # 4
CHUNKS = [8, 20, 20, 16, 16, 16, 8, 4, 4]
CH_MAX = max(CHUNKS)
RAMP = 3          # chunks whose G1 loads ride SWDGE, before any stores
TAIL = len(CHUNKS) - 3

_compiled = None


def _build():
    N = N_PER_CORE
    dt = mybir.dt.float32
    dt16 = mybir.dt.float16
    nc = bacc.Bacc("TRN2", target_bir_lowering=False, debug=False)
    xr = nc.dram_tensor("xr", [N, 64, H, W], dt16, kind="ExternalInput").ap()
    xm = nc.dram_tensor("xm", [N, 56, H, W], dt, kind="ExternalInput").ap()
    y = nc.dram_tensor("y", [N, C_OUT, H, W], dt16, kind="ExternalOutput").ap()

    FM = CH_MAX * W
    ge, mul = mybir.AluOpType.is_ge, mybir.AluOpType.mult
    n_chunks = len(CHUNKS)
    h0s = [sum(CHUNKS[:i]) for i in range(n_chunks)]

    def ring_a(ci):
        return nc.sync if ci % 2 == 0 else nc.scalar

    def ring_b(ci):
        return nc.scalar if ci % 2 == 0 else nc.sync

    with tile.TileContext(nc) as tc, ExitStack() as ctx:
        p1 = ctx.enter_context(tc.tile_pool(name="g1", bufs=4))
        p2 = ctx.enter_context(tc.tile_pool(name="g2", bufs=5))
        p3 = ctx.enter_context(tc.tile_pool(name="g3", bufs=5))
        o1 = ctx.enter_context(tc.tile_pool(name="o1", bufs=3))
        o2 = ctx.enter_context(tc.tile_pool(name="o2", bufs=3))
        o3 = ctx.enter_context(tc.tile_pool(name="o3", bufs=3))
        tp = ctx.enter_context(tc.tile_pool(name="tmp", bufs=1))

        x1t, x2t, x3t = {}, {}, {}

        def issue_x1(ci, eng_a, eng_b):
            ch = CHUNKS[ci]
            hs = slice(h0s[ci], h0s[ci] + ch)
            F = ch * W
            xa = p1.tile([128, FM], dt16, tag="a")
            eng_a.dma_start(
                out=xa[:, :F],
                in_=xr[0:2, :, hs, :].rearrange("n c h w -> c n (h w)"))
            xb = p1.tile([128, FM], dt16, tag="b")
            eng_b.dma_start(
                out=xb[:, :F],
                in_=xr[2:4, :, hs, :].rearrange("n c h w -> c n (h w)"))
            x1t[ci] = (xa, xb)

        def issue_x23(ci):
            ch = CHUNKS[ci]
            hs = slice(h0s[ci], h0s[ci] + ch)
            F = ch * W
            x2 = p2.tile([128, FM], dt)
            ring_a(ci).dma_start(
                out=x2[:, :F],
                in_=xm[:, 0:32, hs, :].rearrange("n c h w -> c n (h w)"))
            x2t[ci] = x2
            x3 = p3.tile([96, FM], dt)
            ring_b(ci).dma_start(
                out=x3[:, :F],
                in_=xm[:, 32:56, hs, :].rearrange("n c h w -> c n (h w)"))
            x3t[ci] = x3

        # Ramp: G1 loads for chunks 0..RAMP-1 on SWDGE, ahead of any store.
        for ci in range(RAMP):
            issue_x1(ci, nc.gpsimd, nc.gpsimd)

        for ci, ch in enumerate(CHUNKS):
            h0 = h0s[ci]
            hs = slice(h0, h0 + ch)
            F = ch * W
            if ci < TAIL:
                issue_x23(ci)
                if ci >= RAMP:
                    issue_x1(ci, ring_b(ci), ring_a(ci))
                st1a = st1b = st2 = st3 = nc.gpsimd
            else:
                if ci == TAIL:
                    for cj in range(TAIL, n_chunks):
                        issue_x23(cj)
                        issue_x1(cj, ring_b(cj), ring_a(cj))
                st1a = st1b = nc.gpsimd
                st2, st3 = ring_a(ci), ring_b(ci)

            x1a, x1b = x1t.pop(ci)
            x2 = x2t.pop(ci)
            x3 = x3t.pop(ci)

            # ---- G1 relu on ACT (f16 in -> f16 out) ----
            for x1, ns, tg, st in ((x1a, slice(0, 2), "a", st1a),
                                   (x1b, slice(2, 4), "b", st1b)):
                y1 = o1.tile([128, FM], dt16, tag=tg)
                nc.scalar.activation(
                    y1[:, :F], x1[:, :F], mybir.ActivationFunctionType.Relu
                )
                st.dma_start(
                    out=y[ns, 0:64, hs, :].rearrange("n c h w -> c n (h w)"),
                    in_=y1[:, :F],
                )

            # ---- G2: 2x2 blocks, channels [64,96) ----
            x2v = x2[:, :F].rearrange("p (h w) -> p h w", h=ch)
            s1 = tp.tile([128, CH_MAX * (W // 2)], dt, tag="s1")
            s1v = s1[:, : ch * (W // 2)].rearrange("p (h w) -> p h w", h=ch)
            nc.vector.tensor_add(s1v, x2v[:, :, 0::2], x2v[:, :, 1::2])
            s2 = tp.tile([128, (CH_MAX // 2) * (W // 2)], dt, tag="s2")
            s2v = s2[:, : (ch // 2) * (W // 2)].rearrange(
                "p (h w) -> p h w", h=ch // 2)
            nc.vector.tensor_add(s2v, s1v[:, 0::2, :], s1v[:, 1::2, :])
            y2 = o2.tile([128, FM], dt16)
            y2v = y2[:, :F].rearrange("p (h w) -> p h w", h=ch)
            m2 = s2v.broadcast_to([128, ch // 2, W // 2, 2])
            for i in range(2):
                nc.vector.scalar_tensor_tensor(
                    y2v[:, i::2, :].rearrange("p h (w j) -> p h w j", j=2),
                    m2, 0.0,
                    x2v[:, i::2, :].rearrange("p h (w j) -> p h w j", j=2),
                    ge, mul,
                )
            st2.dma_start(
                out=y[:, 64:96, hs, :].rearrange("n c h w -> c n (h w)"),
                in_=y2[:, :F],
            )

            # ---- G3: 4x4 blocks, channels [96,120) ----
            x3v = x3[:, :F].rearrange("p (h w) -> p h w", h=ch)
            t1 = tp.tile([96, CH_MAX * (W // 2)], dt, tag="t1")
            t1v = t1[:, : ch * (W // 2)].rearrange("p (h w) -> p h w", h=ch)
            nc.vector.tensor_add(t1v, x3v[:, :, 0::2], x3v[:, :, 1::2])
            t2 = tp.tile([96, CH_MAX * (W // 4)], dt, tag="t2")
            t2v = t2[:, : ch * (W // 4)].rearrange("p (h w) -> p h w", h=ch)
            nc.vector.tensor_add(t2v, t1v[:, :, 0::2], t1v[:, :, 1::2])
            t3 = tp.tile([96, (CH_MAX // 2) * (W // 4)], dt, tag="t3")
            t3v = t3[:, : (ch // 2) * (W // 4)].rearrange(
                "p (h w) -> p h w", h=ch // 2)
            nc.vector.tensor_add(t3v, t2v[:, 0::2, :], t2v[:, 1::2, :])
            t4 = tp.tile([96, (CH_MAX // 4) * (W // 4)], dt, tag="t4")
            t4v = t4[:, : (ch // 4) * (W // 4)].rearrange(
                "p (h w) -> p h w", h=ch // 4)
            nc.vector.tensor_add(t4v, t3v[:, 0::2, :], t3v[:, 1::2, :])
            y3 = o3.tile([96, FM], dt16)
            y3v = y3[:, :F].rearrange("p (h w) -> p h w", h=ch)
            m3 = t4v.broadcast_to([96, ch // 4, W // 4, 4])
            for i in range(4):
                nc.vector.scalar_tensor_tensor(
                    y3v[:, i::4, :].rearrange("p h (w j) -> p h w j", j=4),
                    m3, 0.0,
                    x3v[:, i::4, :].rearrange("p h (w j) -> p h w j", j=4),
                    ge, mul,
                )
            st3.dma_start(
                out=y[:, 96:120, hs, :].rearrange("n c h w -> c n (h w)"),
                in_=y3[:, :F],
            )

    nc.compile()
    return nc


def _get_compiled():
    global _compiled
    if _compiled is None:
        _compiled = _build()
    return _compiled


def kernel(activation: np.ndarray, _trace: bool = False):
    nc = _get_compiled()
    activation = np.ascontiguousarray(activation, dtype=np.float32)
    xr_full = activation[:, 0:64].astype(np.float16)
    in_maps = []
    for i in range(N_CORES):
        n0 = i * N_PER_CORE
        in_maps.append({
            "xr": xr_full[n0 : n0 + N_PER_CORE],
            "xm": np.ascontiguousarray(
                activation[n0 : n0 + N_PER_CORE, 64:C_OUT]),
        })
    res = run_bass_kernel_spmd(nc, in_maps, core_ids=list(range(N_CORES)),
                               trace=_trace)
    out = np.empty((N_FULL, C, H, W), dtype=np.float32)
    for i, r in enumerate(res.results):
        n0 = i * N_PER_CORE
        out[n0 : n0 + N_PER_CORE, :C_OUT] = r["y"].astype(np.float32)
        out[n0 : n0 + N_PER_CORE, C_OUT:] = activation[n0 : n0 + N_PER_CORE, C_OUT:]
    if _trace:
        return out, res
    return out


# revision 10
# speedup vs baseline: 1.1787x; 1.1787x over previous
"""BlockReLU Trainium2 kernel (v5).

Full input: activation [32, 128, 112, 112] f32. Channel groups:
  [0,64): 1x1 blocks (plain ReLU), [64,96): 2x2 blocks, [96,120): 4x4 blocks,
  [120,128): identity passthrough.
A block's mask is 1 where the block's spatial sum >= 0, else 0; broadcast over
the block and multiplied into the input.

Data-parallel over batch N across 8 cores (4 images/core), H streamed in
chunks. Traffic: G1 loads fp16 (rounding never flips sign(x) so the 1x1 mask
is exact), G2/G3 loads fp32 (lossy inputs would flip near-zero block-sum
signs; sum tree matches the reference bit-for-bit), all stores fp16 via
engine write-port conversion, identity channels handled host-side.
Per-core 29.7 MB vs 51.4 baseline. Measured per-core DMA ceiling ~420 GB/s
(16 engines x 26 GB/s, cost linear in packet size).

Scheduling (from v4 trace analysis):
  - Chunk taper [8,20,20,16,16,16,8,4,4]: small first chunk starts compute
    early, big chunks 1-2 cut descriptor-push count while the pipe fills,
    4-row last chunks drain the compute+store tail fast.
  - Ramp: chunks 0-2 G1 loads ride the otherwise-idle SWDGE ring, emitted
    BEFORE any store instruction so they are never head-of-line blocked by
    a store's semaphore wait; HWDGE rings then carry only x2/x3 early.
  - Tail (last 3 chunks): ALL remaining loads are emitted before any tail
    store per engine program (v4 put stores ahead of later loads on the
    same ring, serializing the tail); y2/y3 stores flush on the HWDGE
    rings, y1 on SWDGE - 3-way parallel drain.
  - Mid-stream: loads on both HWDGE rings (x2 vs x3+x1 split alternates by
    parity for byte balance), stores on SWDGE only.
DVE: mask apply per row-parity plane - 3D-collapsible APs, contiguous fp16
writes, stride-0 broadcast of the mask over the w-block (STT is limited to
3D; 4D/5D forms that don't collapse are rejected by the compiler).
History: v1 148/135 us, v2 (fp16 stores) 120, v3 (bf16 G1, no identity) 103,
v4 (taper, fp16 G1, plane STT) 98.8, v5 predicted ~90.
"""
import sys

if "/opt/trn_rl_repo" not in sys.path:
    sys.path.insert(0, "/opt/trn_rl_repo")

import numpy as np
from contextlib import ExitStack

import concourse.tile as tile
from concourse import bacc, mybir
from concourse.bass_utils import run_bass_kernel_spmd

N_FULL, C, H, W = 32, 128, 112, 112
C_OUT = 120
N_CORES = 8
N_PER_CORE = N_FULL // N_CORES  # 4
CHUNKS = [8, 16, 16, 16, 16, 16, 16, 4, 4]
CH_MAX = max(CHUNKS)
RAMP = 0          # v5's SWDGE-ramp loads regressed; disabled
TAIL = len(CHUNKS) - 2

_compiled = None


def _build():
    N = N_PER_CORE
    dt = mybir.dt.float32
    dt16 = mybir.dt.float16
    nc = bacc.Bacc("TRN2", target_bir_lowering=False, debug=False)
    xr = nc.dram_tensor("xr", [N, 64, H, W], dt16, kind="ExternalInput").ap()
    xm = nc.dram_tensor("xm", [N, 56, H, W], dt, kind="ExternalInput").ap()
    y = nc.dram_tensor("y", [N, C_OUT, H, W], dt16, kind="ExternalOutput").ap()

    FM = CH_MAX * W
    ge, mul = mybir.AluOpType.is_ge, mybir.AluOpType.mult
    n_chunks = len(CHUNKS)
    h0s = [sum(CHUNKS[:i]) for i in range(n_chunks)]

    def ring_a(ci):
        return nc.sync if ci % 2 == 0 else nc.scalar

    def ring_b(ci):
        return nc.scalar if ci % 2 == 0 else nc.sync

    with tile.TileContext(nc) as tc, ExitStack() as ctx:
        p1 = ctx.enter_context(tc.tile_pool(name="g1", bufs=4))
        p2 = ctx.enter_context(tc.tile_pool(name="g2", bufs=4))
        p3 = ctx.enter_context(tc.tile_pool(name="g3", bufs=4))
        o1 = ctx.enter_context(tc.tile_pool(name="o1", bufs=4))
        o2 = ctx.enter_context(tc.tile_pool(name="o2", bufs=4))
        o3 = ctx.enter_context(tc.tile_pool(name="o3", bufs=4))
        tp = ctx.enter_context(tc.tile_pool(name="tmp", bufs=1))

        x1t, x2t, x3t = {}, {}, {}

        def issue_x1(ci, eng_a, eng_b):
            ch = CHUNKS[ci]
            hs = slice(h0s[ci], h0s[ci] + ch)
            F = ch * W
            xa = p1.tile([128, FM], dt16, tag="a")
            eng_a.dma_start(
                out=xa[:, :F],
                in_=xr[0:2, :, hs, :].rearrange("n c h w -> c n (h w)"))
            xb = p1.tile([128, FM], dt16, tag="b")
            eng_b.dma_start(
                out=xb[:, :F],
                in_=xr[2:4, :, hs, :].rearrange("n c h w -> c n (h w)"))
            x1t[ci] = (xa, xb)

        def issue_x23(ci):
            ch = CHUNKS[ci]
            hs = slice(h0s[ci], h0s[ci] + ch)
            F = ch * W
            x2 = p2.tile([128, FM], dt)
            ring_a(ci).dma_start(
                out=x2[:, :F],
                in_=xm[:, 0:32, hs, :].rearrange("n c h w -> c n (h w)"))
            x2t[ci] = x2
            x3 = p3.tile([96, FM], dt)
            ring_b(ci).dma_start(
                out=x3[:, :F],
                in_=xm[:, 32:56, hs, :].rearrange("n c h w -> c n (h w)"))
            x3t[ci] = x3

        # Ramp: G1 loads for chunks 0..RAMP-1 on SWDGE, ahead of any store.
        for ci in range(RAMP):
            issue_x1(ci, nc.gpsimd, nc.gpsimd)

        for ci, ch in enumerate(CHUNKS):
            h0 = h0s[ci]
            hs = slice(h0, h0 + ch)
            F = ch * W
            issue_x23(ci)
            issue_x1(ci, ring_b(ci), ring_a(ci))
            if ci < TAIL:
                st1a = st1b = st2 = st3 = nc.gpsimd
            else:
                st1a, st1b = ring_a(ci), ring_b(ci)
                st2, st3 = ring_a(ci), ring_b(ci)

            x1a, x1b = x1t.pop(ci)
            x2 = x2t.pop(ci)
            x3 = x3t.pop(ci)

            # ---- G1 relu on ACT (f16 in -> f16 out) ----
            for x1, ns, tg, st in ((x1a, slice(0, 2), "a", st1a),
                                   (x1b, slice(2, 4), "b", st1b)):
                y1 = o1.tile([128, FM], dt16, tag=tg)
                nc.scalar.activation(
                    y1[:, :F], x1[:, :F], mybir.ActivationFunctionType.Relu
                )
                st.dma_start(
                    out=y[ns, 0:64, hs, :].rearrange("n c h w -> c n (h w)"),
                    in_=y1[:, :F],
                )

            # ---- G2: 2x2 blocks, channels [64,96) ----
            x2v = x2[:, :F].rearrange("p (h w) -> p h w", h=ch)
            s1 = tp.tile([128, CH_MAX * (W // 2)], dt, tag="s1")
            s1v = s1[:, : ch * (W // 2)].rearrange("p (h w) -> p h w", h=ch)
            nc.vector.tensor_add(s1v, x2v[:, :, 0::2], x2v[:, :, 1::2])
            s2 = tp.tile([128, (CH_MAX // 2) * (W // 2)], dt, tag="s2")
            s2v = s2[:, : (ch // 2) * (W // 2)].rearrange(
                "p (h w) -> p h w", h=ch // 2)
            nc.vector.tensor_add(s2v, s1v[:, 0::2, :], s1v[:, 1::2, :])
            y2 = o2.tile([128, FM], dt16)
            y2v = y2[:, :F].rearrange("p (h w) -> p h w", h=ch)
            m2 = s2v.broadcast_to([128, ch // 2, W // 2, 2])
            for i in range(2):
                nc.vector.scalar_tensor_tensor(
                    y2v[:, i::2, :].rearrange("p h (w j) -> p h w j", j=2),
                    m2, 0.0,
                    x2v[:, i::2, :].rearrange("p h (w j) -> p h w j", j=2),
                    ge, mul,
                )
            st2.dma_start(
                out=y[:, 64:96, hs, :].rearrange("n c h w -> c n (h w)"),
                in_=y2[:, :F],
            )

            # ---- G3: 4x4 blocks, channels [96,120) ----
            x3v = x3[:, :F].rearrange("p (h w) -> p h w", h=ch)
            t1 = tp.tile([96, CH_MAX * (W // 2)], dt, tag="t1")
            t1v = t1[:, : ch * (W // 2)].rearrange("p (h w) -> p h w", h=ch)
            nc.vector.tensor_add(t1v, x3v[:, :, 0::2], x3v[:, :, 1::2])
            t2 = tp.tile([96, CH_MAX * (W // 4)], dt, tag="t2")
            t2v = t2[:, : ch * (W // 4)].rearrange("p (h w) -> p h w", h=ch)
            nc.vector.tensor_add(t2v, t1v[:, :, 0::2], t1v[:, :, 1::2])
            t3 = tp.tile([96, (CH_MAX // 2) * (W // 4)], dt, tag="t3")
            t3v = t3[:, : (ch // 2) * (W // 4)].rearrange(
                "p (h w) -> p h w", h=ch // 2)
            nc.vector.tensor_add(t3v, t2v[:, 0::2, :], t2v[:, 1::2, :])
            t4 = tp.tile([96, (CH_MAX // 4) * (W // 4)], dt, tag="t4")
            t4v = t4[:, : (ch // 4) * (W // 4)].rearrange(
                "p (h w) -> p h w", h=ch // 4)
            nc.vector.tensor_add(t4v, t3v[:, 0::2, :], t3v[:, 1::2, :])
            y3 = o3.tile([96, FM], dt16)
            y3v = y3[:, :F].rearrange("p (h w) -> p h w", h=ch)
            m3 = t4v.broadcast_to([96, ch // 4, W // 4, 4])
            for i in range(4):
                nc.vector.scalar_tensor_tensor(
                    y3v[:, i::4, :].rearrange("p h (w j) -> p h w j", j=4),
                    m3, 0.0,
                    x3v[:, i::4, :].rearrange("p h (w j) -> p h w j", j=4),
                    ge, mul,
                )
            st3.dma_start(
                out=y[:, 96:120, hs, :].rearrange("n c h w -> c n (h w)"),
                in_=y3[:, :F],
            )

    nc.compile()
    return nc


def _get_compiled():
    global _compiled
    if _compiled is None:
        _compiled = _build()
    return _compiled


def kernel(activation: np.ndarray, _trace: bool = False):
    nc = _get_compiled()
    activation = np.ascontiguousarray(activation, dtype=np.float32)
    xr_full = activation[:, 0:64].astype(np.float16)
    in_maps = []
    for i in range(N_CORES):
        n0 = i * N_PER_CORE
        in_maps.append({
            "xr": xr_full[n0 : n0 + N_PER_CORE],
            "xm": np.ascontiguousarray(
                activation[n0 : n0 + N_PER_CORE, 64:C_OUT]),
        })
    res = run_bass_kernel_spmd(nc, in_maps, core_ids=list(range(N_CORES)),
                               trace=_trace)
    out = np.empty((N_FULL, C, H, W), dtype=np.float32)
    for i, r in enumerate(res.results):
        n0 = i * N_PER_CORE
        out[n0 : n0 + N_PER_CORE, :C_OUT] = r["y"].astype(np.float32)
        out[n0 : n0 + N_PER_CORE, C_OUT:] = activation[n0 : n0 + N_PER_CORE, C_OUT:]
    if _trace:
        return out, res
    return out
